# revision 3
# baseline (speedup 1.0000x reference)
"""Bahdanau additive attention on 8 TRN2 NeuronCores.

reference:
    proj    = einsum('bsh,oh->bso', encoder_outputs, W)        # [b,s,o]
    scores  = einsum('bso,o->bs', tanh(proj + dec[:,None,:]), v)
    weights = softmax(scores, -1)                              # [b,s]
    context = einsum('bs,bsh->bh', weights, encoder_outputs)   # [b,h]
    return (context, weights)

Strategy: data-parallel over batch (32 batches -> 8 cores x 4). No
collectives needed; gather on host. Per core, all matmuls run in float32r
(TF32-like: 4x faster than fp32 on the PE at ~1.5e-4 relative error).

Per 512-token chunk of a batch:
  - DMA encoder chunk in natural [s_p, h_f] layout (f32r passthrough)
  - PE-transpose to [h_p, s_f] tiles (needed: PE contracts along partitions)
  - proj: 8 o-tiles x 8 k-tiles of [128,128]x[128,512] f32r matmuls
  - ACT fused tanh(proj + dec): dec is a per-partition bias column
  - scores: v-column matmuls [128,1].T @ tanh[128,512] accumulated in PSUM
  - exp without max-subtraction (scores are O(1)-bounded; safe in fp32),
    with fused running sum via ACT accum_out
  - context: exp-column matmuls vs the natural-layout encoder chunk,
    accumulated in PSUM across the whole batch
Batch epilogue: total = sum of chunk sums, reciprocal, scale weights and
context, DMA out.
"""
import numpy as np

import concourse.bass as bass
import concourse.tile as tile
from concourse import bacc, mybir
from concourse import bass_utils

N_CORES = 8
BATCH = 32
B_CORE = BATCH // N_CORES  # 4
S = 2048
H = 1024
O = 1024
P = 128
S_CHUNK = 512
N_CHUNKS = S // S_CHUNK           # 4
ST = S_CHUNK // P                 # s-tiles per chunk: 4
KT = H // P                       # contraction tiles: 8
OT = O // P                       # output-hidden tiles: 8
NH = H // 512                     # context free-dim chunks: 2

F32 = mybir.dt.float32
F32R = mybir.dt.float32r
TANH = mybir.ActivationFunctionType.Tanh
EXP = mybir.ActivationFunctionType.Exp


def build():
    nc = bacc.Bacc("TRN2", target_bir_lowering=False, debug=False)

    enc = nc.dram_tensor("enc", [B_CORE, S, H], F32R, kind="ExternalInput")
    wt = nc.dram_tensor("wt", [H, O], F32R, kind="ExternalInput")       # W.T
    vt = nc.dram_tensor("vt", [P, OT], F32R, kind="ExternalInput")      # v cols
    dect = nc.dram_tensor("dect", [P, B_CORE * OT], F32, kind="ExternalInput")
    ctx_o = nc.dram_tensor("context", [B_CORE, O], F32, kind="ExternalOutput")
    wts_o = nc.dram_tensor("weights", [B_CORE, S], F32, kind="ExternalOutput")
    ident = nc.inline_tensor(np.eye(P, dtype=np.float32), name="ident")

    with tile.TileContext(nc) as tc:
        with (
            tc.tile_pool(name="const", bufs=1) as const_pool,
            tc.tile_pool(name="encp", bufs=3) as enc_pool,
            tc.tile_pool(name="encTp", bufs=2) as encT_pool,
            tc.tile_pool(name="tanhp", bufs=3) as tanh_pool,
            tc.tile_pool(name="expp", bufs=2) as exp_pool,
            tc.tile_pool(name="smallp", bufs=2) as small_pool,
            tc.tile_pool(name="outp", bufs=2) as out_pool,
            tc.tile_pool(name="ps_proj", bufs=2, space="PSUM") as ps_proj_pool,
            tc.tile_pool(name="ps_tr", bufs=2, space="PSUM") as ps_tr_pool,
            tc.tile_pool(name="ps_sc", bufs=1, space="PSUM") as ps_sc_pool,
            tc.tile_pool(name="ps_ctx", bufs=1, space="PSUM") as ps_ctx_pool,
        ):
            # one-time constants
            wt_sb = const_pool.tile([P, KT, OT, P], F32R, tag="wt")
            nc.sync.dma_start(
                out=wt_sb[:],
                in_=wt.ap().rearrange("(kt p) (ot f) -> p kt ot f", p=P, f=P),
            )
            vt_sb = const_pool.tile([P, OT], F32R, tag="vt")
            nc.sync.dma_start(out=vt_sb[:], in_=vt.ap()[:, :])
            dect_sb = const_pool.tile([P, B_CORE * OT], F32, tag="dect")
            nc.sync.dma_start(out=dect_sb[:], in_=dect.ap()[:, :])
            id_f32 = const_pool.tile([P, P], F32, tag="idf")
            nc.sync.dma_start(out=id_f32[:], in_=ident.ap()[:, :])
            id_sb = const_pool.tile([P, P], F32R, tag="id")
            nc.vector.tensor_copy(id_sb[:], id_f32[:])

            for b in range(B_CORE):
                exp_sb = exp_pool.tile([1, S], F32, tag="exp")
                sums = small_pool.tile([1, N_CHUNKS], F32, tag="sums")
                ps_ctx = ps_ctx_pool.tile([1, NH, 512], F32, tag="ctx")

                for c in range(N_CHUNKS):
                    enc_t = enc_pool.tile([P, ST, H], F32R, tag="enc")
                    nc.sync.dma_start(
                        out=enc_t[:],
                        in_=enc.ap()[b, c * S_CHUNK:(c + 1) * S_CHUNK, :]
                        .rearrange("(st p) h -> p st h", p=P),
                    )

                    # transpose chunk to [h_p, s_f]
                    encT = encT_pool.tile([P, KT, S_CHUNK], F32R, tag="encT")
                    for ht in range(KT):
                        ps_tr = ps_tr_pool.tile([P, S_CHUNK], F32R, tag="tr")
                        for st in range(ST):
                            nc.tensor.transpose(
                                ps_tr[:, st * P:(st + 1) * P],
                                enc_t[:, st, ht * P:(ht + 1) * P],
                                id_sb[:],
                            )
                        nc.vector.tensor_copy(encT[:, ht, :], ps_tr[:])

                    # proj -> tanh -> scores
                    ps_s = ps_sc_pool.tile([1, S_CHUNK], F32, tag="sc")
                    for ot in range(OT):
                        ps_p = ps_proj_pool.tile([P, S_CHUNK], F32, tag="proj")
                        for kt in range(KT):
                            nc.tensor.matmul(
                                ps_p[:, :],
                                wt_sb[:, kt, ot, :],
                                encT[:, kt, :],
                                start=(kt == 0),
                                stop=(kt == KT - 1),
                            )
                        th = tanh_pool.tile([P, S_CHUNK], F32R, tag="tanh")
                        nc.scalar.activation(
                            th[:], ps_p[:, :], TANH,
                            bias=dect_sb[:, b * OT + ot: b * OT + ot + 1],
                        )
                        nc.tensor.matmul(
                            ps_s[:, :],
                            vt_sb[:, ot:ot + 1],
                            th[:],
                            start=(ot == 0),
                            stop=(ot == OT - 1),
                            skip_group_check=True,
                        )

                    # exp (no max subtraction) + fused chunk sum
                    nc.scalar.activation(
                        exp_sb[:, c * S_CHUNK:(c + 1) * S_CHUNK], ps_s[:, :],
                        EXP, accum_out=sums[:, c:c + 1],
                    )

                    # transpose exp row chunk into [128,1] columns
                    ps_e = ps_sc_pool.tile([P, ST], F32, tag="exptr")
                    for st in range(ST):
                        nc.tensor.transpose(
                            ps_e[:, st:st + 1],
                            exp_sb[0:1, c * S_CHUNK + st * P: c * S_CHUNK + (st + 1) * P],
                            id_f32[0:1, 0:1],
                        )
                    expT = small_pool.tile([P, ST], F32R, tag="expT")
                    nc.vector.tensor_copy(expT[:], ps_e[:])

                    # context accumulation vs natural-layout chunk
                    for st in range(ST):
                        for nh in range(NH):
                            nc.tensor.matmul(
                                ps_ctx[:, nh, :],
                                expT[:, st:st + 1],
                                enc_t[:, st, nh * 512:(nh + 1) * 512],
                                start=(c == 0 and st == 0),
                                stop=(c == N_CHUNKS - 1 and st == ST - 1),
                                skip_group_check=True,
                            )

                # batch epilogue
                tot = small_pool.tile([1, 1], F32, tag="tot")
                nc.vector.tensor_reduce(
                    tot[:], sums[:], axis=mybir.AxisListType.X,
                    op=mybir.AluOpType.add,
                )
                rec = small_pool.tile([1, 1], F32, tag="rec")
                nc.vector.reciprocal(rec[:], tot[:])

                wts_sb = out_pool.tile([1, S], F32, tag="wts")
                nc.vector.tensor_scalar_mul(wts_sb[:], exp_sb[:], rec[:])
                nc.sync.dma_start(out=wts_o.ap()[b:b + 1, :], in_=wts_sb[:])

                ctx_sb = out_pool.tile([1, O], F32, tag="ctx")
                nc.vector.tensor_scalar_mul(
                    ctx_sb[:], ps_ctx[:].rearrange("p nh f -> p (nh f)"), rec[:]
                )
                nc.sync.dma_start(out=ctx_o.ap()[b:b + 1, :], in_=ctx_sb[:])

    nc.compile()
    return nc


_NC_CACHE = None


def _get_nc():
    global _NC_CACHE
    if _NC_CACHE is None:
        _NC_CACHE = build()
    return _NC_CACHE


def _make_in_maps(decoder_state, encoder_outputs, W, v):
    decoder_state = np.ascontiguousarray(decoder_state, dtype=np.float32)
    encoder_outputs = np.ascontiguousarray(encoder_outputs, dtype=np.float32)
    W = np.ascontiguousarray(W, dtype=np.float32)
    v = np.ascontiguousarray(v, dtype=np.float32)

    wt = np.ascontiguousarray(W.T)                       # [H, O]
    vt = np.ascontiguousarray(v.reshape(OT, P).T)        # [P, OT]

    in_maps = []
    for i in range(N_CORES):
        dec_sh = decoder_state[i * B_CORE:(i + 1) * B_CORE]          # [4, O]
        dect = np.ascontiguousarray(
            dec_sh.reshape(B_CORE, OT, P).transpose(2, 0, 1).reshape(P, B_CORE * OT)
        )
        in_maps.append({
            "enc": encoder_outputs[i * B_CORE:(i + 1) * B_CORE],
            "wt": wt,
            "vt": vt,
            "dect": dect,
        })
    return in_maps


def run(decoder_state, encoder_outputs, W, v, trace=False):
    nc = _get_nc()
    in_maps = _make_in_maps(decoder_state, encoder_outputs, W, v)
    res = bass_utils.run_bass_kernel_spmd(
        nc, in_maps, core_ids=list(range(N_CORES)), trace=trace,
    )
    context = np.concatenate([res.results[i]["context"] for i in range(N_CORES)], axis=0)
    weights = np.concatenate([res.results[i]["weights"] for i in range(N_CORES)], axis=0)
    return (context, weights), res


def kernel(decoder_state, encoder_outputs, W, v):
    (context, weights), _ = run(decoder_state, encoder_outputs, W, v, trace=False)
    return (context, weights)
